# revision 11
# baseline (speedup 1.0000x reference)
"""Trainium2 Bass kernel for nn_LinearReferenceEnergy (histogram_binning).

out[g] = sum_{a in graph g (64 consecutive atoms)} weight[0, atom_types[a]]

Sharding: data-parallel across 8 NeuronCores; core i gets atoms
[i*65536, (i+1)*65536) == graphs [i*1024, (i+1)*1024); weight replicated.

Per-core (raw Bass, single basic block):
  t[128, 512] int32 <- DMA   (partition p = graphs [8p, 8p+8))
  eq[128, 118, 8, 64] bf16 = (t16 == type_iota)  via one broadcast tensor_tensor
  in-place halving-tree over the 64-atom axis -> counts (exact ints <= 64)
  out[p, s] = sum_c w[c] * cnt[p, c, s]  (mult + reduce, f32)
"""

import sys

import numpy as np

if "/opt/trn_rl_repo" not in sys.path:
    sys.path.insert(0, "/opt/trn_rl_repo")

import concourse.bass as bass
from concourse import mybir
from concourse.bass_utils import run_bass_kernel_spmd

N_CORES = 8
N_TYPES = 118
N_GRAPHS = 8192
ATOMS_PER_GRAPH = 64
N_ATOMS = N_GRAPHS * ATOMS_PER_GRAPH  # 524288

A_CORE = N_ATOMS // N_CORES   # 65536
G_CORE = N_GRAPHS // N_CORES  # 1024
P = 128
F = A_CORE // P               # 512
S = F // ATOMS_PER_GRAPH      # 8

_BUILT = None


def _build():
    nc = bass.Bass("TRN2", target_bir_lowering=False, debug=False)
    t_d = nc.dram_tensor("t_in", [A_CORE], mybir.dt.int32, kind="ExternalInput")
    w_d = nc.dram_tensor("w_in", [1, N_TYPES], mybir.dt.float32, kind="ExternalInput")
    o_d = nc.dram_tensor("out", [G_CORE], mybir.dt.float32, kind="ExternalOutput")

    i16 = mybir.dt.int16
    f32 = mybir.dt.float32
    bf16 = mybir.dt.bfloat16

    t32 = nc.alloc_sbuf_tensor("t32", [P, F], mybir.dt.int32).ap()
    t16 = nc.alloc_sbuf_tensor("t16", [P, F], i16).ap()
    ctypes = nc.alloc_sbuf_tensor("ctypes", [P, N_TYPES], i16).ap()
    wsb = nc.alloc_sbuf_tensor("wsb", [P, N_TYPES], f32).ap()
    eq = nc.alloc_sbuf_tensor("eq", [P, N_TYPES, S, ATOMS_PER_GRAPH], bf16).ap()
    cntf = nc.alloc_sbuf_tensor("cntf", [P, N_TYPES, S], f32).ap()
    prod = nc.alloc_sbuf_tensor("prod", [P, S, N_TYPES], f32).ap()
    osb = nc.alloc_sbuf_tensor("osb", [P, S], f32).ap()

    with (
        nc.Block() as block,
        nc.semaphore("s_in") as s_in,
        nc.semaphore("s_io") as s_io,
        nc.semaphore("s_vec") as s_vec,
        nc.semaphore("s_out") as s_out,
    ):

        @block.sync
        def _(sync: bass.BassEngine):
            sync.dma_start(
                out=t32, in_=t_d.ap().rearrange("(p f) -> p f", p=P)
            ).then_inc(s_in, 16)
            sync.dma_start(out=wsb, in_=w_d.ap().partition_broadcast(P)).then_inc(
                s_in, 16
            )
            sync.wait_ge(s_out, 16)

        C_GP = 0
        C_DVE = N_TYPES
        gp_steps = [1]

        @block.gpsimd
        def _(g: bass.BassEngine):
            g.iota(
                ctypes, pattern=[[1, N_TYPES]], base=0, channel_multiplier=0
            ).then_inc(s_io, 1)

        vec_steps = [0]

        @block.vector
        def _(v: bass.BassEngine):
            v.wait_ge(s_in, 32)
            v.wait_ge(s_io, 1)

            def step(ins):
                vec_steps[0] += 1
                ins.then_inc(s_vec, 1)
                v.wait_ge(s_vec, vec_steps[0])

            step(v.tensor_copy(t16, t32))
            t_b = (
                t16.rearrange("p (s i) -> p s i", s=S)
                .unsqueeze(1)
                .broadcast_to([P, C_DVE, S, ATOMS_PER_GRAPH])
            )
            c_b = (
                ctypes[:, 0:C_DVE]
                .unsqueeze(2)
                .unsqueeze(3)
                .broadcast_to([P, C_DVE, S, ATOMS_PER_GRAPH])
            )
            step(
                v.tensor_tensor(
                    out=eq[:, 0:C_DVE], in0=t_b, in1=c_b, op=mybir.AluOpType.is_equal
                )
            )
            w_ = ATOMS_PER_GRAPH
            while w_ > 1:
                h = w_ // 2
                step(
                    v.tensor_tensor(
                        out=eq[:, 0:C_DVE, :, 0:h],
                        in0=eq[:, 0:C_DVE, :, 0:h],
                        in1=eq[:, 0:C_DVE, :, h:w_],
                        op=mybir.AluOpType.add,
                    )
                )
                w_ = h
            step(v.tensor_copy(cntf[:, 0:C_DVE], eq[:, 0:C_DVE, :, 0:1].squeeze(3)))
            # merge: wait for the gpsimd share of cntf
            v.wait_ge(s_io, gp_steps[0])
            step(
                v.tensor_tensor(
                    out=prod,
                    in0=cntf.rearrange("p c s -> p s c"),
                    in1=wsb.unsqueeze(1).broadcast_to([P, S, N_TYPES]),
                    op=mybir.AluOpType.mult,
                )
            )
            v.tensor_reduce(
                out=osb, in_=prod, axis=mybir.AxisListType.X, op=mybir.AluOpType.add
            ).then_inc(s_vec, 1)
            vec_steps[0] += 1

        @block.scalar
        def _(sc: bass.BassEngine):
            sc.wait_ge(s_vec, vec_steps[0])
            sc.dma_start(
                out=o_d.ap().rearrange("(p s) -> p s", p=P), in_=osb
            ).then_inc(s_out, 16)

    return nc


def _get_nc():
    global _BUILT
    if _BUILT is None:
        _BUILT = _build()
    return _BUILT


def _make_in_maps(atom_types, weight):
    t = np.ascontiguousarray(np.asarray(atom_types))
    if t.dtype != np.int32:
        t = t.astype(np.int32)
    w = np.ascontiguousarray(np.asarray(weight, dtype=np.float32)).reshape(1, N_TYPES)
    t = t.reshape(N_CORES, A_CORE)
    return [{"t_in": t[i], "w_in": w} for i in range(N_CORES)]


def _run(atom_types, weight, trace=False, **kwargs):
    nc = _get_nc()
    in_maps = _make_in_maps(atom_types, weight)
    res = run_bass_kernel_spmd(
        nc, in_maps, core_ids=list(range(N_CORES)), trace=trace, **kwargs
    )
    out = np.concatenate([np.asarray(res.results[i]["out"]) for i in range(N_CORES)])
    return out.reshape(N_GRAPHS, 1).astype(np.float32), res


def kernel(atom_types, n_node, weight):
    n = np.asarray(n_node)
    assert n.shape == (N_GRAPHS,) and np.all(n == ATOMS_PER_GRAPH), (
        "kernel hardcodes 64 atoms per graph"
    )
    out, _ = _run(atom_types, weight, trace=False)
    return out


def run_profiled(atom_types, n_node, weight, **kwargs):
    """Returns (output, BassKernelResults) with NTFF trace/exec_time_ns."""
    return _run(atom_types, weight, trace=True, **kwargs)


# revision 13
# speedup vs baseline: 1.0977x; 1.0977x over previous
"""Trainium2 Bass kernel for nn_LinearReferenceEnergy (histogram_binning).

out[g] = sum_{a in graph g (64 consecutive atoms)} weight[0, atom_types[a]]

Sharding: data-parallel across 8 NeuronCores; core i gets atoms
[i*65536, (i+1)*65536) == graphs [i*1024, (i+1)*1024); weight replicated.

Per-core (raw Bass, single basic block):
  t[128, 512] int32 <- DMA   (partition p = graphs [8p, 8p+8))
  eq[128, 118, 8, 64] bf16 = (t16 == type_iota)  via one broadcast tensor_tensor
  in-place halving-tree over the 64-atom axis -> counts (exact ints <= 64)
  out[p, s] = sum_c w[c] * cnt[p, c, s]  (mult + reduce, f32)
"""

import sys

import numpy as np

if "/opt/trn_rl_repo" not in sys.path:
    sys.path.insert(0, "/opt/trn_rl_repo")

import concourse.bass as bass
from concourse import mybir
from concourse.bass_utils import run_bass_kernel_spmd

N_CORES = 8
N_TYPES = 118
N_GRAPHS = 8192
ATOMS_PER_GRAPH = 64
N_ATOMS = N_GRAPHS * ATOMS_PER_GRAPH  # 524288

A_CORE = N_ATOMS // N_CORES   # 65536
G_CORE = N_GRAPHS // N_CORES  # 1024
P = 128
F = A_CORE // P               # 512
S = F // ATOMS_PER_GRAPH      # 8

_BUILT = None


def _build():
    nc = bass.Bass("TRN2", target_bir_lowering=False, debug=False)
    t_d = nc.dram_tensor("t_in", [A_CORE], mybir.dt.int32, kind="ExternalInput")
    w_d = nc.dram_tensor("w_in", [1, N_TYPES], mybir.dt.float32, kind="ExternalInput")
    o_d = nc.dram_tensor("out", [G_CORE], mybir.dt.float32, kind="ExternalOutput")

    i16 = mybir.dt.int16
    f32 = mybir.dt.float32
    bf16 = mybir.dt.bfloat16

    t32 = nc.alloc_sbuf_tensor("t32", [P, F], mybir.dt.int32).ap()
    t16 = nc.alloc_sbuf_tensor("t16", [P, F], i16).ap()
    ctypes = nc.alloc_sbuf_tensor("ctypes", [P, N_TYPES], i16).ap()
    wsb = nc.alloc_sbuf_tensor("wsb", [P, N_TYPES], f32).ap()
    eq = nc.alloc_sbuf_tensor("eq", [P, N_TYPES, S, ATOMS_PER_GRAPH], bf16).ap()
    prod = nc.alloc_sbuf_tensor("prod", [P, S, N_TYPES], f32).ap()
    osb = nc.alloc_sbuf_tensor("osb", [P, S], f32).ap()

    with (
        nc.Block() as block,
        nc.semaphore("s_in") as s_in,
        nc.semaphore("s_io") as s_io,
        nc.semaphore("s_vec") as s_vec,
        nc.semaphore("s_out") as s_out,
    ):

        @block.sync
        def _(sync: bass.BassEngine):
            sync.dma_start(
                out=t32, in_=t_d.ap().rearrange("(p f) -> p f", p=P)
            ).then_inc(s_in, 16)
            sync.dma_start(out=wsb, in_=w_d.ap().partition_broadcast(P)).then_inc(
                s_in, 16
            )
            sync.wait_ge(s_out, 16)

        C_GP = 0
        C_DVE = N_TYPES
        gp_steps = [1]

        @block.gpsimd
        def _(g: bass.BassEngine):
            g.iota(
                ctypes, pattern=[[1, N_TYPES]], base=0, channel_multiplier=0
            ).then_inc(s_io, 1)

        vec_steps = [0]

        @block.vector
        def _(v: bass.BassEngine):
            v.wait_ge(s_in, 32)
            v.wait_ge(s_io, 1)

            def step(ins):
                vec_steps[0] += 1
                ins.then_inc(s_vec, 1)
                v.wait_ge(s_vec, vec_steps[0])

            step(v.tensor_copy(t16, t32))
            t_b = (
                t16.rearrange("p (s i) -> p s i", s=S)
                .unsqueeze(1)
                .broadcast_to([P, C_DVE, S, ATOMS_PER_GRAPH])
            )
            c_b = (
                ctypes[:, 0:C_DVE]
                .unsqueeze(2)
                .unsqueeze(3)
                .broadcast_to([P, C_DVE, S, ATOMS_PER_GRAPH])
            )
            step(
                v.tensor_tensor(
                    out=eq[:, 0:C_DVE], in0=t_b, in1=c_b, op=mybir.AluOpType.is_equal
                )
            )
            w_ = ATOMS_PER_GRAPH
            while w_ > 1:
                h = w_ // 2
                step(
                    v.tensor_tensor(
                        out=eq[:, 0:C_DVE, :, 0:h],
                        in0=eq[:, 0:C_DVE, :, 0:h],
                        in1=eq[:, 0:C_DVE, :, h:w_],
                        op=mybir.AluOpType.add,
                    )
                )
                w_ = h
            step(
                v.tensor_tensor(
                    out=prod,
                    in0=eq[:, :, :, 0:1].squeeze(3).rearrange("p c s -> p s c"),
                    in1=wsb.unsqueeze(1).broadcast_to([P, S, N_TYPES]),
                    op=mybir.AluOpType.mult,
                )
            )
            v.tensor_reduce(
                out=osb, in_=prod, axis=mybir.AxisListType.X, op=mybir.AluOpType.add
            ).then_inc(s_vec, 1)
            vec_steps[0] += 1

        @block.scalar
        def _(sc: bass.BassEngine):
            sc.wait_ge(s_vec, vec_steps[0])
            sc.dma_start(
                out=o_d.ap().rearrange("(p s) -> p s", p=P), in_=osb
            ).then_inc(s_out, 16)

    return nc


def _get_nc():
    global _BUILT
    if _BUILT is None:
        _BUILT = _build()
    return _BUILT


def _make_in_maps(atom_types, weight):
    t = np.ascontiguousarray(np.asarray(atom_types))
    if t.dtype != np.int32:
        t = t.astype(np.int32)
    w = np.ascontiguousarray(np.asarray(weight, dtype=np.float32)).reshape(1, N_TYPES)
    t = t.reshape(N_CORES, A_CORE)
    return [{"t_in": t[i], "w_in": w} for i in range(N_CORES)]


def _run(atom_types, weight, trace=False, **kwargs):
    nc = _get_nc()
    in_maps = _make_in_maps(atom_types, weight)
    res = run_bass_kernel_spmd(
        nc, in_maps, core_ids=list(range(N_CORES)), trace=trace, **kwargs
    )
    out = np.concatenate([np.asarray(res.results[i]["out"]) for i in range(N_CORES)])
    return out.reshape(N_GRAPHS, 1).astype(np.float32), res


def kernel(atom_types, n_node, weight):
    n = np.asarray(n_node)
    assert n.shape == (N_GRAPHS,) and np.all(n == ATOMS_PER_GRAPH), (
        "kernel hardcodes 64 atoms per graph"
    )
    out, _ = _run(atom_types, weight, trace=False)
    return out


def run_profiled(atom_types, n_node, weight, **kwargs):
    """Returns (output, BassKernelResults) with NTFF trace/exec_time_ns."""
    return _run(atom_types, weight, trace=True, **kwargs)
